# revision 6
# baseline (speedup 1.0000x reference)
"""Head-sharded (tensor-parallel) CrossAttention kernel for 8 trn2 NeuronCores.

Problem shapes (hardcoded): B=2, L=2048, QD=1024, H=16, D=64.
Each core owns 2 heads end-to-end (proj -> QK-RMSNorm -> RoPE -> attention
-> output projection partial); the all-reduce over cores happens on host.

Per-core dataflow (all matmuls in float32r: fp32 bits, 1 cycle/row on PE):
  Phase P: x^T tiles (stationary) x Wqkv^T (moving) -> qkv [bl,384] in PSUM;
           QK RMS-norm stats via ACT Square+accum; RoPE on DVE with
           host-precomputed coefficient planes (q_scale/k_scale folded in);
           q-hat/k-hat transposed on PE into [m, bl]; v staged as [bl, d|1]
           augmented with a ones column (yields softmax denominator for free).
  Phase A: scores^T chunk = k-hat^T slice x q-hat^T (K=64) -> PSUM;
           exp via ACT with per-partition scale = rrms_k/sqrt(D) (k-norm is
           folded into the exp argument); o^T accum = [v|1]^T x expS (K=128);
           PSUM row 64 = denominator; normalize via DVE recip + GPSIMD
           partition-broadcast + DVE mul.
  Phase O: out[bl,e] partial = o^T (stationary) x Wproj^T slice (moving);
           evacuate PSUM to bf16 and DMA out.  Host sums the 8 partials in
           fp32 and adds bproj.
"""

import numpy as np

import concourse.bass as bass
import concourse.tile as tile
from concourse import bacc, mybir
from concourse.bass_utils import run_bass_kernel_spmd

F32 = mybir.dt.float32
F32R = mybir.dt.float32r
BF16 = mybir.dt.bfloat16
AF = mybir.ActivationFunctionType

B, L, QD, H, D = 2, 2048, 1024, 16, 64
INNER = H * D
NCORES = 8
HL = H // NCORES          # heads per core = 2
M = HL * D                # 128 head-dim rows per core
BL = B * L                # 4096
NT = BL // 128            # 32 bl-tiles
NTH = NT // 2             # 16 tiles per sub-phase
CT = QD // 128            # 8 contraction tiles
LQC = 512                 # lq chunk
NLQ = L // LQC            # 4
NLK = L // 128            # 16 lk chunks

_CACHE = {}


def _build_nc():
    nc = bacc.Bacc("TRN2", target_bir_lowering=False, debug=False)

    # Per-core DRAM tensors (host supplies per-core slices).
    xt = nc.dram_tensor("xt", [CT, 128, BL], F32R, kind="ExternalInput")       # x^T tiled [ci, c, bl]
    wqkv = nc.dram_tensor("wqkv", [CT, 128, 3 * M], F32R, kind="ExternalInput")  # [ci, c, q|k|v]
    wproj = nc.dram_tensor("wproj", [M, QD], F32R, kind="ExternalInput")
    coefs = nc.dram_tensor("coefs", [L, 4 * M], F32, kind="ExternalInput")    # [l, plane(4) x grp(4) x d2(32)]
    outp = nc.dram_tensor("outp", [BL, QD], BF16, kind="ExternalOutput")

    with tile.TileContext(nc) as tc:
        with (
            tc.tile_pool(name="res", bufs=1) as res,          # resident
            tc.tile_pool(name="xs", bufs=3) as xs,            # x streaming
            tc.tile_pool(name="cf", bufs=3) as cf,            # coef streaming
            tc.tile_pool(name="stg", bufs=2) as stg,          # qkv staging (per sub-phase)
            tc.tile_pool(name="wk", bufs=3) as wk,            # small work tiles
            tc.tile_pool(name="es", bufs=5) as esp,           # exp(scores) tiles
            tc.tile_pool(name="ob", bufs=4) as obp,           # output staging
            tc.tile_pool(name="pA", bufs=3, space="PSUM") as pA,   # [128,512] psum
            tc.tile_pool(name="pB", bufs=2, space="PSUM") as pB,   # o psum
            tc.tile_pool(name="pT", bufs=2, space="PSUM") as pT,   # transpose psum
        ):
            # ---- resident tensors ----
            wqkv_sb = res.tile([128, CT, 3 * M], F32R)
            for ci in range(CT):
                nc.sync.dma_start(out=wqkv_sb[:, ci, :], in_=wqkv[ci, :, :])
            wproj_sb = res.tile([M, QD], F32R)
            nc.sync.dma_start(out=wproj_sb, in_=wproj[:, :])

            qhT = res.tile([M, BL], F32R)        # q-hat^T  [m, bl]
            khT = res.tile([M, BL], F32R)        # k-hat^T  [m, bl]
            vaug = res.tile([128, NT, 2 * (D + 1)], F32R)  # [bl%128, tile, vh0|1|vh1|1]
            oT = res.tile([M, BL], F32R)         # normalized o^T
            rr = res.tile([128, NT, 4], F32)    # rrms: q_h0,q_h1 | (k_h0,k_h1)/8

            ones_f = res.tile([128, NT], F32)
            nc.vector.memset(ones_f, 1.0)
            nc.vector.tensor_copy(
                vaug[:, :, D : D + 1].rearrange("p a b -> p (a b)"), ones_f
            )
            nc.vector.tensor_copy(
                vaug[:, :, 2 * D + 1 : 2 * D + 2].rearrange("p a b -> p (a b)"), ones_f
            )

            from concourse.masks import make_identity
            ident_f = res.tile([128, 128], F32)
            make_identity(nc, ident_f)
            identr = res.tile([128, 128], F32R)
            nc.vector.tensor_copy(identr, ident_f)

            eps_q = res.tile([128, 1], F32)
            nc.vector.memset(eps_q, 1e-6)
            eps_k = res.tile([128, 1], F32)
            nc.vector.memset(eps_k, float(D) * 1e-6)

            # ================= Phase P =================
            for sp in range(2):  # sub-phase over halves of bl
                qkv_st = stg.tile([128, NTH, 3 * M], F32, tag="stage")
                ssq = stg.tile([128, NTH, 4], F32, tag="ssq")
                for jj in range(NTH):
                    j = sp * NTH + jj
                    xt_t = xs.tile([128, CT, 128], F32R)
                    for ci in range(CT):
                        nc.sync.dma_start(
                            out=xt_t[:, ci, :],
                            in_=xt[ci, :, j * 128 : (j + 1) * 128],
                        )
                    ps = pA.tile([128, 3 * M], F32, tag="big")
                    for ci in range(CT):
                        nc.tensor.matmul(
                            ps,
                            lhsT=xt_t[:, ci, :],
                            rhs=wqkv_sb[:, ci, :],
                            start=(ci == 0),
                            stop=(ci == CT - 1),
                        )
                    # sum of squares per head group (q_h0,q_h1,k_h0,k_h1)
                    for g in range(4):
                        sqs = wk.tile([128, D], F32, tag="sqscratch")
                        nc.scalar.activation(
                            out=sqs,
                            in_=ps[:, g * D : (g + 1) * D],
                            func=AF.Square,
                            accum_out=ssq[:, jj, g : g + 1],
                        )
                    # stage qkv to SBUF
                    nc.vector.tensor_copy(qkv_st[:, jj, :], ps)
                    # v columns -> augmented stationary layout
                    nc.vector.tensor_copy(vaug[:, j, 0:D], ps[:, 2 * M : 2 * M + D])
                    nc.vector.tensor_copy(
                        vaug[:, j, D + 1 : 2 * D + 1], ps[:, 2 * M + D : 3 * M]
                    )

                # rrms for q (plain) and k (pre-divided by sqrt(D)=8)
                sqr = stg.tile([128, NTH, 4], F32, tag="sqr")
                nc.scalar.activation(
                    out=sqr[:, :, 0:2], in_=ssq[:, :, 0:2], func=AF.Sqrt,
                    scale=1.0 / D, bias=eps_q[:, 0:1],
                )
                nc.scalar.activation(
                    out=sqr[:, :, 2:4], in_=ssq[:, :, 2:4], func=AF.Sqrt,
                    scale=1.0, bias=eps_k[:, 0:1],
                )
                nc.vector.reciprocal(
                    rr[:, sp * NTH : (sp + 1) * NTH, :], sqr
                )

                for jj in range(NTH):
                    j = sp * NTH + jj
                    # normalize q in place (k-norm folded into exp scale later)
                    for g in range(2):
                        nc.vector.tensor_scalar_mul(
                            qkv_st[:, jj, g * D : (g + 1) * D],
                            qkv_st[:, jj, g * D : (g + 1) * D],
                            rr[:, j, g : g + 1],
                        )
                    # RoPE over q|k (4 groups x 32 pairs), coef planes A,B,C,D
                    cft = cf.tile([128, 4 * M], F32)
                    lrow = (j % NLK) * 128
                    nc.sync.dma_start(out=cft, in_=coefs[lrow : lrow + 128, :])

                    qk = wk.tile([128, 2 * M], F32R, tag="ropeout")
                    t1 = wk.tile([128, M], F32, tag="ropetmp")
                    src = qkv_st[:, jj, 0 : 2 * M].rearrange(
                        "p (g d2 two) -> p g d2 two", g=4, two=2
                    )
                    dst = qk.rearrange("p (g d2 two) -> p g d2 two", g=4, two=2)
                    pl = [
                        cft[:, i * M : (i + 1) * M].rearrange(
                            "p (g d2) -> p g d2", g=4
                        )
                        for i in range(4)
                    ]
                    t1v = t1.rearrange("p (g d2) -> p g d2", g=4)
                    ev, od = src[:, :, :, 0], src[:, :, :, 1]
                    # out_even = A*ev + B*od ; out_odd = C*ev + D*od
                    nc.vector.tensor_mul(dst[:, :, :, 0], ev, pl[0])
                    nc.vector.tensor_mul(t1v, od, pl[1])
                    nc.vector.tensor_add(dst[:, :, :, 0], dst[:, :, :, 0], t1v)
                    nc.vector.tensor_mul(dst[:, :, :, 1], ev, pl[2])
                    nc.vector.tensor_mul(t1v, od, pl[3])
                    nc.vector.tensor_add(dst[:, :, :, 1], dst[:, :, :, 1], t1v)

                    # transpose q,k tiles into [m, bl] residents
                    for which, dest in ((0, qhT), (1, khT)):
                        pst = pT.tile([128, 128], F32R, tag="tp")
                        nc.tensor.transpose(
                            pst,
                            qk[:, which * M : (which + 1) * M],
                            identr,
                        )
                        nc.vector.tensor_copy(
                            dest[:, j * 128 : (j + 1) * 128], pst
                        )

            # ================= Phase A =================
            for b in range(B):
                for h in range(HL):
                    for lq in range(NLQ):
                        qs = qhT[
                            h * D : (h + 1) * D,
                            b * L + lq * LQC : b * L + (lq + 1) * LQC,
                        ]
                        po = pB.tile([D + 1, LQC], F32, tag="o")
                        for lk in range(NLK):
                            j = b * NLK + lk
                            pss = pA.tile([128, LQC], F32, tag="big")
                            nc.tensor.matmul(
                                pss,
                                lhsT=khT[
                                    h * D : (h + 1) * D,
                                    b * L + lk * 128 : b * L + (lk + 1) * 128,
                                ],
                                rhs=qs,
                                start=True,
                                stop=True,
                            )
                            es = esp.tile([128, LQC], F32R, tag="es")
                            nc.scalar.activation(
                                out=es, in_=pss, func=AF.Exp,
                                scale=rr[:, j, 2 + h : 3 + h],
                            )
                            nc.tensor.matmul(
                                po,
                                lhsT=vaug[
                                    :, j, h * (D + 1) : (h + 1) * (D + 1)
                                ],
                                rhs=es,
                                start=(lk == 0),
                                stop=(lk == NLK - 1),
                                skip_group_check=True,
                            )
                        # normalize: o / denom (denom in row D)
                        rd = wk.tile([1, LQC], F32, tag="rd")
                        nc.vector.reciprocal(rd, po[D : D + 1, :])
                        rdb = wk.tile([D, LQC], F32, tag="rdb")
                        nc.gpsimd.partition_broadcast(rdb, rd)
                        nc.vector.tensor_mul(
                            oT[
                                h * D : (h + 1) * D,
                                b * L + lq * LQC : b * L + (lq + 1) * LQC,
                            ],
                            po[0:D, :],
                            rdb,
                        )

            # ================= Phase O =================
            for j in range(NT):
                for eo in range(2):
                    ps = pA.tile([128, 512], F32, tag="big")
                    nc.tensor.matmul(
                        ps,
                        lhsT=oT[:, j * 128 : (j + 1) * 128],
                        rhs=wproj_sb[:, eo * 512 : (eo + 1) * 512],
                        start=True,
                        stop=True,
                    )
                    ob = obp.tile([128, 512], BF16, tag="ob")
                    if eo == 0:
                        nc.vector.tensor_copy(ob, ps)
                    else:
                        nc.scalar.copy(ob, ps)
                    nc.sync.dma_start(
                        out=outp[j * 128 : (j + 1) * 128, eo * 512 : (eo + 1) * 512],
                        in_=ob,
                    )

    nc.compile()
    return nc


def _prep_inputs(x, pe, Wq, Wkv, Wproj, q_scale, k_scale):
    """Build the 8 per-core input maps."""
    x = np.asarray(x, np.float32)
    xT = np.ascontiguousarray(x.reshape(BL, QD).T)              # [QD, BL]
    xt_tiled = np.ascontiguousarray(xT.reshape(CT, 128, BL))    # [ci, c, bl]

    pe = np.asarray(pe, np.float32)[0, 0]                       # [L, 32, 2, 2]
    qs, ks = np.asarray(q_scale, np.float32), np.asarray(k_scale, np.float32)

    def coef_block(scale):
        se, so = scale[0::2], scale[1::2]                       # [32]
        A = pe[:, :, 0, 0] * se[None, :]
        Bm = pe[:, :, 0, 1] * so[None, :]
        C = pe[:, :, 1, 0] * se[None, :]
        Dm = pe[:, :, 1, 1] * so[None, :]
        return A, Bm, C, Dm

    Aq, Bq, Cq, Dq = coef_block(qs)
    Ak, Bk, Ck, Dk = coef_block(ks)
    coefs = np.empty((L, 4, 4, 32), np.float32)                 # [l, plane, grp, d2]
    for p_i, (cq, ck) in enumerate(((Aq, Ak), (Bq, Bk), (Cq, Ck), (Dq, Dk))):
        coefs[:, p_i, 0] = cq
        coefs[:, p_i, 1] = cq
        coefs[:, p_i, 2] = ck
        coefs[:, p_i, 3] = ck
    coefs = np.ascontiguousarray(coefs.reshape(L, 4 * M))

    Wq = np.asarray(Wq, np.float32)
    Wkv = np.asarray(Wkv, np.float32)
    Wproj = np.asarray(Wproj, np.float32)
    Wk_full, Wv_full = Wkv[:INNER], Wkv[INNER:]

    in_maps = []
    for c in range(NCORES):
        r0, r1 = c * HL * D, (c + 1) * HL * D
        wqkv_c = np.concatenate([Wq[r0:r1], Wk_full[r0:r1], Wv_full[r0:r1]], axis=0)
        # [3M, QD] -> [ci, c(128), 3M]
        wqkv_t = np.ascontiguousarray(wqkv_c.T.reshape(CT, 128, 3 * M))
        wproj_c = np.ascontiguousarray(Wproj[:, r0:r1].T)       # [M, QD]
        in_maps.append(
            {"xt": xt_tiled, "wqkv": wqkv_t, "wproj": wproj_c, "coefs": coefs}
        )
    return in_maps


def kernel(x, pe, Wq, Wkv, Wproj, bproj, q_scale, k_scale):
    if "nc" not in _CACHE:
        _CACHE["nc"] = _build_nc()
    nc = _CACHE["nc"]
    in_maps = _prep_inputs(x, pe, Wq, Wkv, Wproj, q_scale, k_scale)
    res = run_bass_kernel_spmd(nc, in_maps, core_ids=list(range(NCORES)))
    acc = np.zeros((BL, QD), np.float32)
    for c in range(NCORES):
        acc += res.results[c]["outp"].astype(np.float32)
    acc += np.asarray(bproj, np.float32)[None, :]
    return acc.reshape(B, L, QD)


# revision 13
# speedup vs baseline: 1.2952x; 1.2952x over previous
"""Head-sharded (tensor-parallel) CrossAttention kernel for 8 trn2 NeuronCores.

Problem shapes (hardcoded): B=2, L=2048, QD=1024, H=16, D=64.
Each core owns 2 heads end-to-end (proj -> QK-RMSNorm -> RoPE -> attention
-> output projection partial); the all-reduce over cores happens on host.

All matmuls run in float32r (fp32 storage, TF32-like rounding, 1 cycle/row
on the PE for moving sizes >= 256).  Per-core dataflow:

  Phase P(b): x^T tiles (stationary) x Wqkv^T (moving) -> qkv [bl,384] PSUM.
     Sum-of-squares for QK RMS-norm is fused into the PSUM->SBUF staging
     copy (DVE tensor_tensor_reduce).  rrms via one batched ACT Sqrt + DVE
     reciprocal.  RoPE on DVE with host-precomputed coefficient planes
     (q_scale/k_scale folded in; even/odd pairs live in the free dim so all
     ops are full 128-partition).  q-hat/k-hat transposed on PE into
     [m, bl]; v staged as [bl, v|1] with a ones column so the o^T matmul
     also produces the softmax denominator.
  Phase A(b): scores^T [lk=128, lq=1024] = two N=512 matmuls into a 2-bank
     PSUM tile; one wide exp per tile via ACT, with per-partition scale
     rrms_k/sqrt(D) (k's norm is folded into the exp argument - softmax is
     computed without max-subtraction, safe since |scores| < 20).
     o^T accumulates [v|1]^T x expS over lk (K=128); row 64 = denominator;
     normalize = DVE recip + GPSIMD partition-broadcast + DVE mul.
     Phase A(b) overlaps Phase P(b+1) across engines.
  Phase O: out[bl,e] partial = o^T (stationary) x Wproj^T (moving);
     PSUM evacuated to bf16 (split DVE/ACT) and DMA'd out.  Host sums the
     8 partials in fp32 and adds bproj.
"""

import numpy as np

import concourse.bass as bass
import concourse.tile as tile
from concourse import bacc, mybir
from concourse.bass_utils import run_bass_kernel_spmd
from concourse.masks import make_identity

F32 = mybir.dt.float32
F32R = mybir.dt.float32r
BF16 = mybir.dt.bfloat16
AF = mybir.ActivationFunctionType
ALU = mybir.AluOpType

B, L, QD, H, D = 2, 2048, 1024, 16, 64
INNER = H * D
NCORES = 8
HL = H // NCORES          # heads per core = 2
M = HL * D                # 128 head-dim rows per core
BL = B * L                # 4096
NT = BL // 128            # 32 bl-tiles
NTH = NT // 2             # 16 tiles per batch
CT = QD // 128            # 8 contraction tiles
LQC = 1024                # lq chunk (2 psum banks)
NLQ = L // LQC            # 2
NLK = L // 128            # 16 lk chunks

_CACHE = {}


def _build_nc():
    nc = bacc.Bacc("TRN2", target_bir_lowering=False, debug=False)

    xt = nc.dram_tensor("xt", [NT, 128, CT, 128], F32R, kind="ExternalInput")
    wqkv = nc.dram_tensor("wqkv", [128, CT, 3 * M], F32R, kind="ExternalInput")
    wproj = nc.dram_tensor("wproj", [M, QD], F32R, kind="ExternalInput")
    coefs = nc.dram_tensor("coefs", [NLK, 128, 4 * M], F32, kind="ExternalInput")
    outp = nc.dram_tensor("outp", [BL, QD], BF16, kind="ExternalOutput")

    with tile.TileContext(nc) as tc:
        with (
            tc.tile_pool(name="res", bufs=1) as res,
            tc.tile_pool(name="xs", bufs=3) as xs,
            tc.tile_pool(name="cf", bufs=3) as cf,
            tc.tile_pool(name="stg", bufs=2) as stg,
            tc.tile_pool(name="wk", bufs=3) as wk,
            tc.tile_pool(name="nrm", bufs=2) as nrm,
            tc.tile_pool(name="es", bufs=4) as esp,
            tc.tile_pool(name="ob", bufs=3) as obp,
            tc.tile_pool(name="pA", bufs=2, space="PSUM") as pA,   # big: 2 banks/buf
            tc.tile_pool(name="pB", bufs=1, space="PSUM") as pB,   # o: 2 banks
        ):
            # ---- residents ----
            wqkv_sb = res.tile([128, CT, 3 * M], F32R)
            nc.sync.dma_start(out=wqkv_sb, in_=wqkv[:, :, :])
            wproj_sb = res.tile([M, QD], F32R)
            nc.sync.dma_start(out=wproj_sb, in_=wproj[:, :])

            qhT = res.tile([M, BL], F32R)
            khT = res.tile([M, BL], F32R)
            vaug = res.tile([128, NT, 2 * (D + 1)], F32R)
            oT = res.tile([M, BL], F32R)
            rr = res.tile([128, NT, 4], F32)

            ones_f = res.tile([128, NT], F32)
            nc.vector.memset(ones_f, 1.0)
            nc.vector.tensor_copy(
                vaug[:, :, D : D + 1].rearrange("p a b -> p (a b)"), ones_f
            )
            nc.vector.tensor_copy(
                vaug[:, :, 2 * D + 1 : 2 * D + 2].rearrange("p a b -> p (a b)"),
                ones_f,
            )
            ident_f = res.tile([128, 128], F32)
            make_identity(nc, ident_f)
            identr = res.tile([128, 128], F32R)
            nc.vector.tensor_copy(identr, ident_f)
            eps_q = res.tile([128, 1], F32)
            nc.vector.memset(eps_q, 1e-6)
            eps_k = res.tile([128, 1], F32)
            nc.vector.memset(eps_k, float(D) * 1e-6)

            for bb in range(B):
                # ====== Phase P(bb): projections + norm stats + rope ======
                qk_st = stg.tile([128, NTH, 2 * M], F32, tag="stage")
                ssq = stg.tile([128, NTH, 4], F32, tag="ssq")
                for jj in range(NTH):
                    j = bb * NTH + jj
                    xt_t = xs.tile([128, CT, 128], F32R)
                    nc.sync.dma_start(out=xt_t, in_=xt[j, :, :, :])
                    ps = pA.tile([128, 3 * M], F32, tag="proj")
                    for ci in range(CT):
                        nc.tensor.matmul(
                            ps,
                            lhsT=xt_t[:, ci, :],
                            rhs=wqkv_sb[:, ci, :],
                            start=(ci == 0),
                            stop=(ci == CT - 1),
                        )
                    # stage q|k to SBUF; sum-of-squares via ACT Square+accum
                    nc.vector.tensor_copy(qk_st[:, jj, :], ps[:, 0 : 2 * M])
                    sqs = wk.tile([128, 2 * M], F32, tag="sqscratch")
                    for g in range(4):
                        nc.scalar.activation(
                            out=sqs[:, g * D : (g + 1) * D],
                            in_=ps[:, g * D : (g + 1) * D],
                            func=AF.Square,
                            accum_out=ssq[:, jj, g : g + 1],
                        )
                    nc.vector.tensor_copy(vaug[:, j, 0:D], ps[:, 2 * M : 2 * M + D])
                    nc.vector.tensor_copy(
                        vaug[:, j, D + 1 : 2 * D + 1], ps[:, 2 * M + D : 3 * M]
                    )

                sqr = stg.tile([128, NTH, 4], F32, tag="sqr")
                nc.scalar.activation(
                    out=sqr[:, :, 0:2], in_=ssq[:, :, 0:2], func=AF.Sqrt,
                    scale=1.0 / D, bias=eps_q[:, 0:1],
                )
                nc.scalar.activation(
                    out=sqr[:, :, 2:4], in_=ssq[:, :, 2:4], func=AF.Sqrt,
                    scale=1.0, bias=eps_k[:, 0:1],
                )
                nc.vector.reciprocal(rr[:, bb * NTH : (bb + 1) * NTH, :], sqr)

                for jj in range(NTH):
                    j = bb * NTH + jj
                    for g in range(2):  # normalize q in place
                        nc.vector.tensor_scalar_mul(
                            qk_st[:, jj, g * D : (g + 1) * D],
                            qk_st[:, jj, g * D : (g + 1) * D],
                            rr[:, j, g : g + 1],
                        )
                    cft = cf.tile([128, 4 * M], F32)
                    nc.sync.dma_start(out=cft, in_=coefs[jj, :, :])

                    qk = wk.tile([128, 2 * M], F32R, tag="ropeout")
                    t1 = wk.tile([128, M], F32, tag="ropetmp")
                    src = qk_st[:, jj, :].rearrange(
                        "p (g d2 two) -> p g d2 two", g=4, two=2
                    )
                    dst = qk.rearrange("p (g d2 two) -> p g d2 two", g=4, two=2)
                    pl = [
                        cft[:, i * M : (i + 1) * M].rearrange("p (g d2) -> p g d2", g=4)
                        for i in range(4)
                    ]
                    t1v = t1.rearrange("p (g d2) -> p g d2", g=4)
                    ev, od = src[:, :, :, 0], src[:, :, :, 1]
                    nc.vector.tensor_mul(dst[:, :, :, 0], ev, pl[0])
                    nc.vector.tensor_mul(t1v, od, pl[1])
                    nc.vector.tensor_add(dst[:, :, :, 0], dst[:, :, :, 0], t1v)
                    nc.vector.tensor_mul(dst[:, :, :, 1], ev, pl[2])
                    nc.vector.tensor_mul(t1v, od, pl[3])
                    nc.vector.tensor_add(dst[:, :, :, 1], dst[:, :, :, 1], t1v)

                    for which, dest in ((0, qhT), (1, khT)):
                        pst = pA.tile([128, 128], F32R, tag="proj")
                        nc.tensor.transpose(
                            pst, qk[:, which * M : (which + 1) * M], identr
                        )
                        nc.vector.tensor_copy(dest[:, j * 128 : (j + 1) * 128], pst)

                # ====== Phase A(bb): attention ======
                for h in range(HL):
                    for lq in range(NLQ):
                        qs = qhT[
                            h * D : (h + 1) * D,
                            bb * L + lq * LQC : bb * L + (lq + 1) * LQC,
                        ]
                        po = pB.tile([D + 1, LQC], F32, tag="o")
                        for lk in range(NLK):
                            j = bb * NLK + lk
                            pss = pA.tile([128, LQC], F32, tag="big")
                            for half in range(2):
                                nc.tensor.matmul(
                                    pss[:, half * 512 : (half + 1) * 512],
                                    lhsT=khT[
                                        h * D : (h + 1) * D,
                                        bb * L + lk * 128 : bb * L + (lk + 1) * 128,
                                    ],
                                    rhs=qs[:, half * 512 : (half + 1) * 512],
                                    start=True,
                                    stop=True,
                                )
                            es = esp.tile([128, LQC], F32R, tag="es")
                            nc.scalar.activation(
                                out=es, in_=pss, func=AF.Exp,
                                scale=rr[:, j, 2 + h : 3 + h],
                            )
                            for half in range(2):
                                nc.tensor.matmul(
                                    po[:, half * 512 : (half + 1) * 512],
                                    lhsT=vaug[:, j, h * (D + 1) : (h + 1) * (D + 1)],
                                    rhs=es[:, half * 512 : (half + 1) * 512],
                                    start=(lk == 0),
                                    stop=(lk == NLK - 1),
                                    skip_group_check=True,
                                )
                        rd = nrm.tile([1, LQC], F32, tag="rd")
                        nc.vector.reciprocal(rd, po[D : D + 1, :])
                        rdb = nrm.tile([D, LQC], F32, tag="rdb")
                        nc.gpsimd.partition_broadcast(rdb, rd)
                        nc.vector.tensor_mul(
                            oT[
                                h * D : (h + 1) * D,
                                bb * L + lq * LQC : bb * L + (lq + 1) * LQC,
                            ],
                            po[0:D, :],
                            rdb,
                        )

                # ====== Phase O(bb) ======
                for j in range(bb * NTH, (bb + 1) * NTH):
                    ps = pA.tile([128, QD], F32, tag="big")
                    for eo in range(2):
                        nc.tensor.matmul(
                            ps[:, eo * 512 : (eo + 1) * 512],
                            lhsT=oT[:, j * 128 : (j + 1) * 128],
                            rhs=wproj_sb[:, eo * 512 : (eo + 1) * 512],
                            start=True,
                            stop=True,
                        )
                    ob = obp.tile([128, QD], BF16, tag="ob")
                    nc.vector.tensor_copy(ob[:, 0:512], ps[:, 0:512])
                    nc.scalar.copy(ob[:, 512:1024], ps[:, 512:1024])
                    nc.sync.dma_start(out=outp[j * 128 : (j + 1) * 128, :], in_=ob)

    nc.compile()
    return nc


def _prep_inputs(x, pe, Wq, Wkv, Wproj, q_scale, k_scale):
    x = np.asarray(x, np.float32)
    xT = np.ascontiguousarray(x.reshape(BL, QD).T)                    # [QD, BL]
    xtt = np.ascontiguousarray(
        xT.reshape(CT, 128, NT, 128).transpose(2, 1, 0, 3)
    )                                                                 # [NT, p, CT, n]

    pe = np.asarray(pe, np.float32)[0, 0]                             # [L, 32, 2, 2]
    qs, ks = np.asarray(q_scale, np.float32), np.asarray(k_scale, np.float32)

    def planes(scale):
        se, so = scale[0::2], scale[1::2]
        return (
            pe[:, :, 0, 0] * se[None, :],
            pe[:, :, 0, 1] * so[None, :],
            pe[:, :, 1, 0] * se[None, :],
            pe[:, :, 1, 1] * so[None, :],
        )

    pq, pk = planes(qs), planes(ks)
    coefs = np.empty((L, 4, 4, 32), np.float32)                       # [l, plane, grp, d2]
    for p_i in range(4):
        coefs[:, p_i, 0] = pq[p_i]
        coefs[:, p_i, 1] = pq[p_i]
        coefs[:, p_i, 2] = pk[p_i]
        coefs[:, p_i, 3] = pk[p_i]
    coefs = np.ascontiguousarray(coefs.reshape(NLK, 128, 4 * M))

    Wq = np.asarray(Wq, np.float32)
    Wkv = np.asarray(Wkv, np.float32)
    Wproj = np.asarray(Wproj, np.float32)
    Wk_full, Wv_full = Wkv[:INNER], Wkv[INNER:]

    in_maps = []
    for c in range(NCORES):
        r0, r1 = c * M, (c + 1) * M
        wqkv_c = np.concatenate([Wq[r0:r1], Wk_full[r0:r1], Wv_full[r0:r1]], axis=0)
        wqkv_t = np.ascontiguousarray(
            wqkv_c.T.reshape(CT, 128, 3 * M).transpose(1, 0, 2)
        )                                                             # [128, CT, 3M]
        wproj_c = np.ascontiguousarray(Wproj[:, r0:r1].T)             # [M, QD]
        in_maps.append(
            {"xt": xtt, "wqkv": wqkv_t, "wproj": wproj_c, "coefs": coefs}
        )
    return in_maps


def kernel(x, pe, Wq, Wkv, Wproj, bproj, q_scale, k_scale):
    if "nc" not in _CACHE:
        _CACHE["nc"] = _build_nc()
    nc = _CACHE["nc"]
    in_maps = _prep_inputs(x, pe, Wq, Wkv, Wproj, q_scale, k_scale)
    res = run_bass_kernel_spmd(nc, in_maps, core_ids=list(range(NCORES)))
    acc = np.zeros((BL, QD), np.float32)
    for c in range(NCORES):
        acc += res.results[c]["outp"].astype(np.float32)
    acc += np.asarray(bproj, np.float32)[None, :]
    return acc.reshape(B, L, QD)


# revision 15
# speedup vs baseline: 1.4023x; 1.0827x over previous
"""Head-sharded (tensor-parallel) CrossAttention kernel for 8 trn2 NeuronCores.

Problem shapes (hardcoded): B=2, L=2048, QD=1024, H=16, D=64.
Each core owns 2 heads end-to-end (proj -> QK-RMSNorm -> RoPE -> attention
-> output projection partial); the all-reduce over cores happens on host.

All matmuls run in float32r (fp32 storage, TF32-like rounding, 1 cycle/row
on the PE for moving sizes >= 256).  Per-core dataflow:

  Phase P(b): x^T tiles (stationary) x Wqkv^T (moving) -> qkv [bl,384] PSUM.
     Sum-of-squares for QK RMS-norm is fused into the PSUM->SBUF staging
     copy (DVE tensor_tensor_reduce).  rrms via one batched ACT Sqrt + DVE
     reciprocal.  RoPE on DVE with host-precomputed coefficient planes
     (q_scale/k_scale folded in; even/odd pairs live in the free dim so all
     ops are full 128-partition).  q-hat/k-hat transposed on PE into
     [m, bl]; v staged as [bl, v|1] with a ones column so the o^T matmul
     also produces the softmax denominator.
  Phase A(b): scores^T [lk=128, lq=1024] = two N=512 matmuls into a 2-bank
     PSUM tile; one wide exp per tile via ACT, with per-partition scale
     rrms_k/sqrt(D) (k's norm is folded into the exp argument - softmax is
     computed without max-subtraction, safe since |scores| < 20).
     o^T accumulates [v|1]^T x expS over lk (K=128); row 64 = denominator;
     normalize = DVE recip + GPSIMD partition-broadcast + DVE mul.
     Phase A(b) overlaps Phase P(b+1) across engines.
  Phase O: out[bl,e] partial = o^T (stationary) x Wproj^T (moving);
     PSUM evacuated to bf16 (split DVE/ACT) and DMA'd out.  Host sums the
     8 partials in fp32 and adds bproj.
"""

import numpy as np

import concourse.bass as bass
import concourse.tile as tile
from concourse import bacc, mybir
from concourse.bass_utils import run_bass_kernel_spmd
from concourse.masks import make_identity

F32 = mybir.dt.float32
F32R = mybir.dt.float32r
BF16 = mybir.dt.bfloat16
AF = mybir.ActivationFunctionType
ALU = mybir.AluOpType

B, L, QD, H, D = 2, 2048, 1024, 16, 64
INNER = H * D
NCORES = 8
HL = H // NCORES          # heads per core = 2
M = HL * D                # 128 head-dim rows per core
BL = B * L                # 4096
NT = BL // 128            # 32 bl-tiles
NTH = NT // 2             # 16 tiles per batch
CT = QD // 128            # 8 contraction tiles
LQC = 1024                # lq chunk (2 psum banks)
NLQ = L // LQC            # 2
NLK = L // 128            # 16 lk chunks

_CACHE = {}


def _build_nc():
    nc = bacc.Bacc("TRN2", target_bir_lowering=False, debug=False)

    xt = nc.dram_tensor("xt", [NT, 128, CT, 128], F32R, kind="ExternalInput")
    wqkv = nc.dram_tensor("wqkv", [128, CT, 3 * M], F32R, kind="ExternalInput")
    wproj = nc.dram_tensor("wproj", [M, QD], F32R, kind="ExternalInput")
    coefs = nc.dram_tensor("coefs", [NLK, 128, 4 * M], F32, kind="ExternalInput")
    outp = nc.dram_tensor("outp", [BL, QD], BF16, kind="ExternalOutput")

    with tile.TileContext(nc) as tc:
        with (
            tc.tile_pool(name="res", bufs=1) as res,
            tc.tile_pool(name="xs", bufs=3) as xs,
            tc.tile_pool(name="cf", bufs=3) as cf,
            tc.tile_pool(name="stg", bufs=2) as stg,
            tc.tile_pool(name="wk", bufs=3) as wk,
            tc.tile_pool(name="nrm", bufs=2) as nrm,
            tc.tile_pool(name="es", bufs=4) as esp,
            tc.tile_pool(name="ob", bufs=3) as obp,
            tc.tile_pool(name="pA", bufs=2, space="PSUM") as pA,   # big: 2 banks/buf
            tc.tile_pool(name="pB", bufs=1, space="PSUM") as pB,   # o: 2 banks
        ):
            # ---- residents ----
            wqkv_sb = res.tile([128, CT, 3 * M], F32R)
            nc.sync.dma_start(out=wqkv_sb, in_=wqkv[:, :, :])
            wproj_sb = res.tile([M, QD], F32R)
            nc.sync.dma_start(out=wproj_sb, in_=wproj[:, :])

            qhT = res.tile([M, BL], F32R)
            khT = res.tile([M, BL], F32R)
            vaug = res.tile([128, NT, 2 * (D + 1)], F32R)
            oT = res.tile([M, BL], F32R)
            rr = res.tile([128, NT, 4], F32)

            ones_f = res.tile([128, NT], F32)
            nc.vector.memset(ones_f, 1.0)
            nc.vector.tensor_copy(
                vaug[:, :, D : D + 1].rearrange("p a b -> p (a b)"), ones_f
            )
            nc.vector.tensor_copy(
                vaug[:, :, 2 * D + 1 : 2 * D + 2].rearrange("p a b -> p (a b)"),
                ones_f,
            )
            ident_f = res.tile([128, 128], F32)
            make_identity(nc, ident_f)
            identr = res.tile([128, 128], F32R)
            nc.vector.tensor_copy(identr, ident_f)
            magic = res.tile([128, 16], mybir.dt.int32)
            nc.vector.memset(magic, 0x5F3759DF)

            for bb in range(B):
                # ====== Phase P(bb): projections + norm stats + rope ======
                NSB = 4  # tiles per sub-batch
                for sb in range(NTH // NSB):
                    qk_st = stg.tile([128, NSB, 2 * M], F32, tag="stage")
                    ssq = stg.tile([128, NSB, 4], F32, tag="ssq")
                    for t in range(NSB):
                        jj = sb * NSB + t
                        j = bb * NTH + jj
                        xt_t = xs.tile([128, CT, 128], F32R)
                        nc.sync.dma_start(out=xt_t, in_=xt[j, :, :, :])
                        ps = pA.tile([128, 3 * M], F32, tag="proj")
                        for ci in range(CT):
                            nc.tensor.matmul(
                                ps,
                                lhsT=xt_t[:, ci, :],
                                rhs=wqkv_sb[:, ci, :],
                                start=(ci == 0),
                                stop=(ci == CT - 1),
                            )
                        # stage q|k to SBUF; sum-of-squares on DVE
                        nc.vector.tensor_copy(qk_st[:, t, :], ps[:, 0 : 2 * M])
                        sqs = wk.tile([128, 2 * M], F32, tag="sqscratch")
                        nc.vector.tensor_mul(sqs, ps[:, 0 : 2 * M], qk_st[:, t, :])
                        nc.vector.reduce_sum(
                            out=ssq[:, t, :].rearrange("p (a b) -> p a b", b=1),
                            in_=sqs.rearrange("p (a b) -> p a b", a=4),
                            axis=mybir.AxisListType.X,
                        )
                        nc.vector.tensor_copy(vaug[:, j, 0:D], ps[:, 2 * M : 2 * M + D])
                        nc.vector.tensor_copy(
                            vaug[:, j, D + 1 : 2 * D + 1], ps[:, 2 * M + D : 3 * M]
                        )

                    # rrms via Newton rsqrt on DVE (rr cols 2:4 hold rrms_k/8)
                    j0 = bb * NTH + sb * NSB
                    rrs = rr[:, j0 : j0 + NSB, :]
                    nx = wk.tile([128, NSB, 4], F32, tag="nx")
                    nc.vector.tensor_scalar(
                        out=nx[:, :, 0:2], in0=ssq[:, :, 0:2],
                        scalar1=1.0 / D, scalar2=1e-6,
                        op0=ALU.mult, op1=ALU.add,
                    )
                    nc.vector.tensor_scalar(
                        out=nx[:, :, 2:4], in0=ssq[:, :, 2:4],
                        scalar1=1.0, scalar2=float(D) * 1e-6,
                        op0=ALU.mult, op1=ALU.add,
                    )
                    sh = wk.tile([128, NSB, 4], mybir.dt.int32, tag="nsh")
                    nc.vector.tensor_scalar(
                        out=sh, in0=nx.bitcast(mybir.dt.int32), scalar1=1,
                        scalar2=None, op0=ALU.logical_shift_right,
                    )
                    nc.vector.tensor_tensor(
                        out=rrs.bitcast(mybir.dt.int32),
                        in0=magic[:, 0 : NSB * 4].rearrange(
                            "p (a b) -> p a b", b=4
                        ),
                        in1=sh,
                        op=ALU.subtract,
                    )
                    ht = wk.tile([128, NSB, 4], F32, tag="nht")
                    for _ in range(2):  # Newton: y *= 1.5 - 0.5*x*y*y
                        nc.vector.tensor_mul(ht, nx, rrs)
                        nc.vector.tensor_mul(ht, ht, rrs)
                        nc.vector.tensor_scalar(
                            out=ht, in0=ht, scalar1=-0.5, scalar2=1.5,
                            op0=ALU.mult, op1=ALU.add,
                        )
                        nc.vector.tensor_mul(rrs, rrs, ht)

                    for t in range(NSB):
                        jj = sb * NSB + t
                        j = bb * NTH + jj
                        for g in range(2):  # normalize q in place
                            nc.vector.tensor_scalar_mul(
                                qk_st[:, t, g * D : (g + 1) * D],
                                qk_st[:, t, g * D : (g + 1) * D],
                                rr[:, j, g : g + 1],
                            )
                        cft = cf.tile([128, 4 * M], F32)
                        nc.sync.dma_start(out=cft, in_=coefs[jj, :, :])

                        qk = wk.tile([128, 2 * M], F32R, tag="ropeout")
                        t1 = wk.tile([128, M], F32, tag="ropetmp")
                        src = qk_st[:, t, :].rearrange(
                            "p (g d2 two) -> p g d2 two", g=4, two=2
                        )
                        dst = qk.rearrange("p (g d2 two) -> p g d2 two", g=4, two=2)
                        pl = [
                            cft[:, i * M : (i + 1) * M].rearrange(
                                "p (g d2) -> p g d2", g=4
                            )
                            for i in range(4)
                        ]
                        t1v = t1.rearrange("p (g d2) -> p g d2", g=4)
                        ev, od = src[:, :, :, 0], src[:, :, :, 1]
                        nc.vector.tensor_mul(dst[:, :, :, 0], ev, pl[0])
                        nc.vector.tensor_mul(t1v, od, pl[1])
                        nc.vector.tensor_add(dst[:, :, :, 0], dst[:, :, :, 0], t1v)
                        nc.vector.tensor_mul(dst[:, :, :, 1], ev, pl[2])
                        nc.vector.tensor_mul(t1v, od, pl[3])
                        nc.vector.tensor_add(dst[:, :, :, 1], dst[:, :, :, 1], t1v)

                        for which, dest in ((0, qhT), (1, khT)):
                            pst = pA.tile([128, 128], F32R, tag="proj")
                            nc.tensor.transpose(
                                pst, qk[:, which * M : (which + 1) * M], identr
                            )
                            nc.vector.tensor_copy(
                                dest[:, j * 128 : (j + 1) * 128], pst
                            )

                # ====== Phase A(bb): attention ======
                for h in range(HL):
                    for lq in range(NLQ):
                        qs = qhT[
                            h * D : (h + 1) * D,
                            bb * L + lq * LQC : bb * L + (lq + 1) * LQC,
                        ]
                        po = pB.tile([D + 1, LQC], F32, tag="o")
                        for lk in range(NLK):
                            j = bb * NLK + lk
                            pss = pA.tile([128, LQC], F32, tag="big")
                            for half in range(2):
                                nc.tensor.matmul(
                                    pss[:, half * 512 : (half + 1) * 512],
                                    lhsT=khT[
                                        h * D : (h + 1) * D,
                                        bb * L + lk * 128 : bb * L + (lk + 1) * 128,
                                    ],
                                    rhs=qs[:, half * 512 : (half + 1) * 512],
                                    start=True,
                                    stop=True,
                                )
                            es = esp.tile([128, LQC], F32R, tag="es")
                            nc.scalar.activation(
                                out=es, in_=pss, func=AF.Exp,
                                scale=rr[:, j, 2 + h : 3 + h],
                            )
                            for half in range(2):
                                nc.tensor.matmul(
                                    po[:, half * 512 : (half + 1) * 512],
                                    lhsT=vaug[:, j, h * (D + 1) : (h + 1) * (D + 1)],
                                    rhs=es[:, half * 512 : (half + 1) * 512],
                                    start=(lk == 0),
                                    stop=(lk == NLK - 1),
                                    skip_group_check=True,
                                )
                        rd = nrm.tile([1, LQC], F32, tag="rd")
                        nc.vector.reciprocal(rd, po[D : D + 1, :])
                        rdb = nrm.tile([D, LQC], F32, tag="rdb")
                        nc.gpsimd.partition_broadcast(rdb, rd)
                        nc.vector.tensor_mul(
                            oT[
                                h * D : (h + 1) * D,
                                bb * L + lq * LQC : bb * L + (lq + 1) * LQC,
                            ],
                            po[0:D, :],
                            rdb,
                        )

                # ====== Phase O(bb) ======
                for j in range(bb * NTH, (bb + 1) * NTH):
                    ps = pA.tile([128, QD], F32, tag="big")
                    for eo in range(2):
                        nc.tensor.matmul(
                            ps[:, eo * 512 : (eo + 1) * 512],
                            lhsT=oT[:, j * 128 : (j + 1) * 128],
                            rhs=wproj_sb[:, eo * 512 : (eo + 1) * 512],
                            start=True,
                            stop=True,
                        )
                    ob = obp.tile([128, QD], BF16, tag="ob")
                    nc.vector.tensor_copy(ob[:, 0:512], ps[:, 0:512])
                    nc.scalar.copy(ob[:, 512:1024], ps[:, 512:1024])
                    nc.sync.dma_start(out=outp[j * 128 : (j + 1) * 128, :], in_=ob)

    nc.compile()
    return nc


def _prep_inputs(x, pe, Wq, Wkv, Wproj, q_scale, k_scale):
    x = np.asarray(x, np.float32)
    xT = np.ascontiguousarray(x.reshape(BL, QD).T)                    # [QD, BL]
    xtt = np.ascontiguousarray(
        xT.reshape(CT, 128, NT, 128).transpose(2, 1, 0, 3)
    )                                                                 # [NT, p, CT, n]

    pe = np.asarray(pe, np.float32)[0, 0]                             # [L, 32, 2, 2]
    qs, ks = np.asarray(q_scale, np.float32), np.asarray(k_scale, np.float32)

    def planes(scale):
        se, so = scale[0::2], scale[1::2]
        return (
            pe[:, :, 0, 0] * se[None, :],
            pe[:, :, 0, 1] * so[None, :],
            pe[:, :, 1, 0] * se[None, :],
            pe[:, :, 1, 1] * so[None, :],
        )

    pq, pk = planes(qs), planes(ks)
    coefs = np.empty((L, 4, 4, 32), np.float32)                       # [l, plane, grp, d2]
    for p_i in range(4):
        coefs[:, p_i, 0] = pq[p_i]
        coefs[:, p_i, 1] = pq[p_i]
        coefs[:, p_i, 2] = pk[p_i]
        coefs[:, p_i, 3] = pk[p_i]
    coefs = np.ascontiguousarray(coefs.reshape(NLK, 128, 4 * M))

    Wq = np.asarray(Wq, np.float32)
    Wkv = np.asarray(Wkv, np.float32)
    Wproj = np.asarray(Wproj, np.float32)
    Wk_full, Wv_full = Wkv[:INNER], Wkv[INNER:]

    in_maps = []
    for c in range(NCORES):
        r0, r1 = c * M, (c + 1) * M
        wqkv_c = np.concatenate([Wq[r0:r1], Wk_full[r0:r1], Wv_full[r0:r1]], axis=0)
        wqkv_t = np.ascontiguousarray(
            wqkv_c.T.reshape(CT, 128, 3 * M).transpose(1, 0, 2)
        )                                                             # [128, CT, 3M]
        wproj_c = np.ascontiguousarray(Wproj[:, r0:r1].T)             # [M, QD]
        in_maps.append(
            {"xt": xtt, "wqkv": wqkv_t, "wproj": wproj_c, "coefs": coefs}
        )
    return in_maps


def kernel(x, pe, Wq, Wkv, Wproj, bproj, q_scale, k_scale):
    if "nc" not in _CACHE:
        _CACHE["nc"] = _build_nc()
    nc = _CACHE["nc"]
    in_maps = _prep_inputs(x, pe, Wq, Wkv, Wproj, q_scale, k_scale)
    res = run_bass_kernel_spmd(nc, in_maps, core_ids=list(range(NCORES)))
    acc = np.zeros((BL, QD), np.float32)
    for c in range(NCORES):
        acc += res.results[c]["outp"].astype(np.float32)
    acc += np.asarray(bproj, np.float32)[None, :]
    return acc.reshape(B, L, QD)


# revision 27
# speedup vs baseline: 1.4316x; 1.0208x over previous
"""Head-sharded (tensor-parallel) CrossAttention kernel for 8 trn2 NeuronCores.

Problem shapes (hardcoded): B=2, L=2048, QD=1024, H=16, D=64.
Each core owns 2 heads end-to-end (proj -> QK-RMSNorm -> RoPE -> attention
-> output projection partial); the all-reduce over cores happens on host.

All matmuls run in float32r (fp32 storage, TF32-like rounding, 1 cycle/row
on the PE for moving sizes >= 256).  Per-core dataflow:

  Phase P(b): x^T tiles (stationary) x Wqkv^T (moving) -> qkv [bl,384] PSUM.
     Sum-of-squares for QK RMS-norm is fused into the PSUM->SBUF staging
     copy (DVE tensor_tensor_reduce).  rrms via one batched ACT Sqrt + DVE
     reciprocal.  RoPE on DVE with host-precomputed coefficient planes
     (q_scale/k_scale folded in; even/odd pairs live in the free dim so all
     ops are full 128-partition).  q-hat/k-hat transposed on PE into
     [m, bl]; v staged as [bl, v|1] with a ones column so the o^T matmul
     also produces the softmax denominator.
  Phase A(b): scores^T [lk=128, lq=1024] = two N=512 matmuls into a 2-bank
     PSUM tile; one wide exp per tile via ACT, with per-partition scale
     rrms_k/sqrt(D) (k's norm is folded into the exp argument - softmax is
     computed without max-subtraction, safe since |scores| < 20).
     o^T accumulates [v|1]^T x expS over lk (K=128); row 64 = denominator;
     normalize = DVE recip + GPSIMD partition-broadcast + DVE mul.
     Phase A(b) overlaps Phase P(b+1) across engines.
  Phase O: out[bl,e] partial = o^T (stationary) x Wproj^T (moving);
     PSUM evacuated to bf16 (split DVE/ACT) and DMA'd out.  Host sums the
     8 partials in fp32 and adds bproj.
"""

import numpy as np

import concourse.bass as bass
import concourse.tile as tile
from concourse import bacc, mybir
from concourse.bass_utils import run_bass_kernel_spmd
from concourse.masks import make_identity

F32 = mybir.dt.float32
F32R = mybir.dt.float32r
BF16 = mybir.dt.bfloat16
AF = mybir.ActivationFunctionType
ALU = mybir.AluOpType

B, L, QD, H, D = 2, 2048, 1024, 16, 64
INNER = H * D
NCORES = 8
HL = H // NCORES          # heads per core = 2
M = HL * D                # 128 head-dim rows per core
BL = B * L                # 4096
NT = BL // 128            # 32 bl-tiles
NTH = NT // 2             # 16 tiles per batch
CT = QD // 128            # 8 contraction tiles
LQC = 1024                # lq chunk (2 psum banks)
NLQ = L // LQC            # 2
NLK = L // 128            # 16 lk chunks

_CACHE = {}


def _build_nc():
    nc = bacc.Bacc("TRN2", target_bir_lowering=False, debug=False)

    xt = nc.dram_tensor("xt", [NT, 128, CT, 128], F32R, kind="ExternalInput")
    wqkv = nc.dram_tensor("wqkv", [128, CT, 3 * M], F32R, kind="ExternalInput")
    wproj = nc.dram_tensor("wproj", [M, QD], F32R, kind="ExternalInput")
    coefs = nc.dram_tensor("coefs", [NLK, 128, 4 * M], F32, kind="ExternalInput")
    outp = nc.dram_tensor("outp", [BL, QD], BF16, kind="ExternalOutput")

    with tile.TileContext(nc) as tc:
        with (
            tc.tile_pool(name="res", bufs=1) as res,
            tc.tile_pool(name="xs", bufs=3) as xs,
            tc.tile_pool(name="cf", bufs=3) as cf,
            tc.tile_pool(name="stg", bufs=3) as stg,
            tc.tile_pool(name="wk", bufs=3) as wk,
            tc.tile_pool(name="nrm", bufs=2) as nrm,
            tc.tile_pool(name="es", bufs=4) as esp,
            tc.tile_pool(name="ob", bufs=3) as obp,
            tc.tile_pool(name="pA", bufs=2, space="PSUM") as pA,   # big: 2 banks/buf
            tc.tile_pool(name="pB", bufs=1, space="PSUM") as pB,   # o: 2 banks
        ):
            # ---- residents ----
            wqkv_sb = res.tile([128, CT, 3 * M], F32R)
            nc.sync.dma_start(out=wqkv_sb, in_=wqkv[:, :, :])
            wproj_sb = res.tile([M, QD], F32R)
            nc.sync.dma_start(out=wproj_sb, in_=wproj[:, :])

            qhT = res.tile([M, BL], F32R)
            khT = res.tile([M, BL], F32R)
            vaug = res.tile([128, NT, 2 * (D + 1)], F32R)
            oT = res.tile([M, BL], F32R)
            rr = res.tile([128, NT, 4], F32)

            ones_f = res.tile([128, NT], F32)
            nc.vector.memset(ones_f, 1.0)
            nc.vector.tensor_copy(
                vaug[:, :, D : D + 1].rearrange("p a b -> p (a b)"), ones_f
            )
            nc.vector.tensor_copy(
                vaug[:, :, 2 * D + 1 : 2 * D + 2].rearrange("p a b -> p (a b)"),
                ones_f,
            )
            ident_f = res.tile([128, 128], F32)
            make_identity(nc, ident_f)
            identr = res.tile([128, 128], F32R)
            nc.vector.tensor_copy(identr, ident_f)
            magic = res.tile([128, 16], mybir.dt.int32)
            nc.vector.memset(magic, 0x5F3759DF)

            NSB = 4  # tiles per P sub-batch; 4 sub-batches per b

            def emit_P_subbatch(bb, sb):
                qk_st = stg.tile([128, NSB, 2 * M], F32, tag="stage")
                ssq = stg.tile([128, NSB, 4], F32, tag="ssq")
                for t in range(NSB):
                    jj = sb * NSB + t
                    j = bb * NTH + jj
                    xt_t = xs.tile([128, CT, 128], F32R)
                    nc.sync.dma_start(out=xt_t, in_=xt[j, :, :, :])
                    ps = pA.tile([128, 3 * M], F32, tag="proj")
                    for ci in range(CT):
                        nc.tensor.matmul(
                            ps,
                            lhsT=xt_t[:, ci, :],
                            rhs=wqkv_sb[:, ci, :],
                            start=(ci == 0),
                            stop=(ci == CT - 1),
                        )
                    nc.vector.tensor_copy(qk_st[:, t, :], ps[:, 0 : 2 * M])
                    sqs = wk.tile([128, 2 * M], F32, tag="sqscratch")
                    nc.gpsimd.tensor_mul(sqs, qk_st[:, t, :], qk_st[:, t, :])
                    nc.vector.reduce_sum(
                        out=ssq[:, t, :].rearrange("p (a b) -> p a b", b=1),
                        in_=sqs.rearrange("p (a b) -> p a b", a=4),
                        axis=mybir.AxisListType.X,
                    )
                    nc.scalar.copy(vaug[:, j, 0:D], ps[:, 2 * M : 2 * M + D])
                    nc.scalar.copy(
                        vaug[:, j, D + 1 : 2 * D + 1], ps[:, 2 * M + D : 3 * M]
                    )

                # rrms via Newton rsqrt on DVE (rr cols 2:4 hold rrms_k/8)
                j0 = bb * NTH + sb * NSB
                rrs = rr[:, j0 : j0 + NSB, :]
                nx = wk.tile([128, NSB, 4], F32, tag="nx")
                nc.vector.tensor_scalar(
                    out=nx[:, :, 0:2], in0=ssq[:, :, 0:2],
                    scalar1=1.0 / D, scalar2=1e-6, op0=ALU.mult, op1=ALU.add,
                )
                nc.vector.tensor_scalar(
                    out=nx[:, :, 2:4], in0=ssq[:, :, 2:4],
                    scalar1=1.0, scalar2=float(D) * 1e-6, op0=ALU.mult, op1=ALU.add,
                )
                sh = wk.tile([128, NSB, 4], mybir.dt.int32, tag="nsh")
                nc.vector.tensor_scalar(
                    out=sh, in0=nx.bitcast(mybir.dt.int32), scalar1=1,
                    scalar2=None, op0=ALU.logical_shift_right,
                )
                nc.vector.tensor_tensor(
                    out=rrs.bitcast(mybir.dt.int32),
                    in0=magic[:, 0 : NSB * 4].rearrange("p (a b) -> p a b", b=4),
                    in1=sh,
                    op=ALU.subtract,
                )
                ht = wk.tile([128, NSB, 4], F32, tag="nht")
                for _ in range(2):  # y *= 1.5 - 0.5*x*y*y
                    nc.vector.tensor_mul(ht, nx, rrs)
                    nc.vector.tensor_mul(ht, ht, rrs)
                    nc.vector.tensor_scalar(
                        out=ht, in0=ht, scalar1=-0.5, scalar2=1.5,
                        op0=ALU.mult, op1=ALU.add,
                    )
                    nc.vector.tensor_mul(rrs, rrs, ht)

                for t in range(NSB):
                    jj = sb * NSB + t
                    j = bb * NTH + jj
                    for g in range(2):  # normalize q in place
                        nc.gpsimd.tensor_scalar_mul(
                            qk_st[:, t, g * D : (g + 1) * D],
                            qk_st[:, t, g * D : (g + 1) * D],
                            rr[:, j, g : g + 1],
                        )
                    cft = cf.tile([128, 4 * M], F32)
                    nc.sync.dma_start(out=cft, in_=coefs[jj, :, :])

                    qk = wk.tile([128, 2 * M], F32R, tag="ropeout")
                    t1 = wk.tile([128, M], F32, tag="ropetmp")
                    src = qk_st[:, t, :].rearrange(
                        "p (g d2 two) -> p g d2 two", g=4, two=2
                    )
                    dst = qk.rearrange("p (g d2 two) -> p g d2 two", g=4, two=2)
                    pl = [
                        cft[:, i * M : (i + 1) * M].rearrange(
                            "p (g d2) -> p g d2", g=4
                        )
                        for i in range(4)
                    ]
                    t1v = t1.rearrange("p (g d2) -> p g d2", g=4)
                    ev, od = src[:, :, :, 0], src[:, :, :, 1]
                    nc.vector.tensor_mul(dst[:, :, :, 0], ev, pl[0])
                    nc.vector.tensor_mul(t1v, od, pl[1])
                    nc.vector.tensor_add(dst[:, :, :, 0], dst[:, :, :, 0], t1v)
                    nc.vector.tensor_mul(dst[:, :, :, 1], ev, pl[2])
                    nc.vector.tensor_mul(t1v, od, pl[3])
                    nc.vector.tensor_add(dst[:, :, :, 1], dst[:, :, :, 1], t1v)

                    for which, dest in ((0, qhT), (1, khT)):
                        pst = pA.tile([128, 128], F32R, tag="proj")
                        nc.tensor.transpose(
                            pst, qk[:, which * M : (which + 1) * M], identr
                        )
                        if which == 0:
                            nc.vector.tensor_copy(
                                dest[:, j * 128 : (j + 1) * 128], pst
                            )
                        else:
                            nc.scalar.copy(dest[:, j * 128 : (j + 1) * 128], pst)

            def emit_A_combo(bb, lq, h):
                qs = qhT[
                    h * D : (h + 1) * D,
                    bb * L + lq * LQC : bb * L + (lq + 1) * LQC,
                ]
                po = pB.tile([D + 1, LQC], F32, tag="o")
                for lk in range(NLK):
                    j = bb * NLK + lk
                    pss = pA.tile([128, LQC], F32, tag="big")
                    for half in range(2):
                        nc.tensor.matmul(
                            pss[:, half * 512 : (half + 1) * 512],
                            lhsT=khT[
                                h * D : (h + 1) * D,
                                bb * L + lk * 128 : bb * L + (lk + 1) * 128,
                            ],
                            rhs=qs[:, half * 512 : (half + 1) * 512],
                            start=True,
                            stop=True,
                        )
                    es = esp.tile([128, LQC], F32R, tag="es")
                    nc.scalar.activation(
                        out=es, in_=pss, func=AF.Exp,
                        scale=rr[:, j, 2 + h : 3 + h],
                    )
                    for half in range(2):
                        nc.tensor.matmul(
                            po[:, half * 512 : (half + 1) * 512],
                            lhsT=vaug[:, j, h * (D + 1) : (h + 1) * (D + 1)],
                            rhs=es[:, half * 512 : (half + 1) * 512],
                            start=(lk == 0),
                            stop=(lk == NLK - 1),
                            skip_group_check=True,
                        )
                rd = nrm.tile([1, LQC], F32, tag="rd")
                nc.vector.reciprocal(rd, po[D : D + 1, :])
                rdb = nrm.tile([D, LQC], F32, tag="rdb")
                nc.gpsimd.partition_broadcast(rdb, rd)
                nc.vector.tensor_mul(
                    oT[
                        h * D : (h + 1) * D,
                        bb * L + lq * LQC : bb * L + (lq + 1) * LQC,
                    ],
                    po[0:D, :],
                    rdb,
                )

            def emit_O_chunk(bb, lq):
                ntpc = LQC // 128  # 8 bl-tiles per lq chunk
                for j in range(bb * NTH + lq * ntpc, bb * NTH + (lq + 1) * ntpc):
                    ps = pA.tile([128, QD], F32, tag="big")
                    for eo in range(2):
                        nc.tensor.matmul(
                            ps[:, eo * 512 : (eo + 1) * 512],
                            lhsT=oT[:, j * 128 : (j + 1) * 128],
                            rhs=wproj_sb[:, eo * 512 : (eo + 1) * 512],
                            start=True,
                            stop=True,
                        )
                    ob = obp.tile([128, QD], BF16, tag="ob")
                    nc.vector.tensor_copy(ob[:, 0:512], ps[:, 0:512])
                    nc.scalar.copy(ob[:, 512:1024], ps[:, 512:1024])
                    nc.sync.dma_start(out=outp[j * 128 : (j + 1) * 128, :], in_=ob)

            for bb in range(B):
                for sb in range(4):
                    emit_P_subbatch(bb, sb)
                for h in range(HL):
                    for lq in range(NLQ):
                        emit_A_combo(bb, lq, h)
                for lq in range(NLQ):
                    emit_O_chunk(bb, lq)

    nc.compile()
    return nc


def _prep_inputs(x, pe, Wq, Wkv, Wproj, q_scale, k_scale):
    x = np.asarray(x, np.float32)
    xT = np.ascontiguousarray(x.reshape(BL, QD).T)                    # [QD, BL]
    xtt = np.ascontiguousarray(
        xT.reshape(CT, 128, NT, 128).transpose(2, 1, 0, 3)
    )                                                                 # [NT, p, CT, n]

    pe = np.asarray(pe, np.float32)[0, 0]                             # [L, 32, 2, 2]
    qs, ks = np.asarray(q_scale, np.float32), np.asarray(k_scale, np.float32)

    def planes(scale):
        se, so = scale[0::2], scale[1::2]
        return (
            pe[:, :, 0, 0] * se[None, :],
            pe[:, :, 0, 1] * so[None, :],
            pe[:, :, 1, 0] * se[None, :],
            pe[:, :, 1, 1] * so[None, :],
        )

    pq, pk = planes(qs), planes(ks)
    coefs = np.empty((L, 4, 4, 32), np.float32)                       # [l, plane, grp, d2]
    for p_i in range(4):
        coefs[:, p_i, 0] = pq[p_i]
        coefs[:, p_i, 1] = pq[p_i]
        coefs[:, p_i, 2] = pk[p_i]
        coefs[:, p_i, 3] = pk[p_i]
    coefs = np.ascontiguousarray(coefs.reshape(NLK, 128, 4 * M))

    Wq = np.asarray(Wq, np.float32)
    Wkv = np.asarray(Wkv, np.float32)
    Wproj = np.asarray(Wproj, np.float32)
    Wk_full, Wv_full = Wkv[:INNER], Wkv[INNER:]

    in_maps = []
    for c in range(NCORES):
        r0, r1 = c * M, (c + 1) * M
        wqkv_c = np.concatenate([Wq[r0:r1], Wk_full[r0:r1], Wv_full[r0:r1]], axis=0)
        wqkv_t = np.ascontiguousarray(
            wqkv_c.T.reshape(CT, 128, 3 * M).transpose(1, 0, 2)
        )                                                             # [128, CT, 3M]
        wproj_c = np.ascontiguousarray(Wproj[:, r0:r1].T)             # [M, QD]
        in_maps.append(
            {"xt": xtt, "wqkv": wqkv_t, "wproj": wproj_c, "coefs": coefs}
        )
    return in_maps


def kernel(x, pe, Wq, Wkv, Wproj, bproj, q_scale, k_scale):
    if "nc" not in _CACHE:
        _CACHE["nc"] = _build_nc()
    nc = _CACHE["nc"]
    in_maps = _prep_inputs(x, pe, Wq, Wkv, Wproj, q_scale, k_scale)
    res = run_bass_kernel_spmd(nc, in_maps, core_ids=list(range(NCORES)))
    acc = np.zeros((BL, QD), np.float32)
    for c in range(NCORES):
        acc += res.results[c]["outp"].astype(np.float32)
    acc += np.asarray(bproj, np.float32)[None, :]
    return acc.reshape(B, L, QD)
